# revision 2
# baseline (speedup 1.0000x reference)
"""Trainium2 Bass kernel: sparse-FFN decode matvec (moe_routing).

out[b, 0, j] = sum_d x[b, 0, d] * weight[indices[j], d]
x [64, 1, 4096] f32, weight [11008, 4096] f32, indices [4403] int64.

Design:
  - dedup indices; quantile sharding: sorted uniques split into 8
    equal-count chunks (q ~= 452); each core ships a fixed W_ROWS-row
    window of the bf16 weight covering its chunk's span.
  - per core: 3 full 128-row tiles via transposed dma_gather (d-major
    SBUF layout), each split along D so matmuls pipeline behind DMA;
    the ~68 leftover rows via ONE exact-size non-transposed gather
    (REM=80 padded to idx-granularity 16) + PE transposes -- saves the
    padded 4th 128-row tile's bytes on the DMA stream.
  - out^T dataflow: gathered d-major chunk is the matmul's stationary
    lhsT [128d x 128n]; xt streams as rhs [128d x 64b] -> psum
    [128n x 64b]; PE time ~halves vs out=[64,128] and Ldweights is free.
  - copies on DVE; all out stores on the otherwise-idle SP HWDGE ring;
    the last row tile ends with a small 512-element D-chunk so the
    dependent tail after the final gather byte is short.
  - output [NPC, 64] f32 (transposed); host transposes/expands.
"""

import numpy as np
import ml_dtypes

V = 11008
D = 4096
B = 64
N_IDX = 4403
NCORES = 8
DC = D // 128          # 32 d-chunks
NFULL = 3              # full 128-row transposed tiles per core
REM = 80               # remainder rows (exact gather, 16-granularity)
NPC = NFULL * 128 + REM
W_ROWS = 1792          # fixed weight-window rows shipped per core
# D-splits: every full tile splits so matmuls pipeline; the final tile
# ends small to shrink the dependent tail.
TILE_SPLIT = (2048, 2048)
LAST_SPLIT = (2048, 1536, 512)
IDX_COLS = NFULL * 8 + REM // 16

_compiled = {}


def _build(w_rows=W_ROWS, reps=1, rem=REM, tile_split=TILE_SPLIT,
           last_split=LAST_SPLIT):
    import concourse.bacc as bacc
    import concourse.bass as bass
    import concourse.mybir as mybir
    import concourse.tile as tile
    from concourse.masks import make_identity

    f32 = mybir.dt.float32
    bf16 = mybir.dt.bfloat16
    i16 = mybir.dt.int16

    npc = NFULL * 128 + rem
    nc = bacc.Bacc(
        "TRN2",
        target_bir_lowering=False,
        debug=False,
        enable_asserts=False,
        num_devices=NCORES,
    )
    w = nc.dram_tensor("w", [w_rows, D], bf16, kind="ExternalInput").ap()
    xt = nc.dram_tensor("xt", [128, DC * B], bf16, kind="ExternalInput").ap()
    idx = nc.dram_tensor("idx", [128, IDX_COLS], i16, kind="ExternalInput").ap()
    out = nc.dram_tensor("out", [npc, B], f32, kind="ExternalOutput").ap()

    # gather schedule for the full tiles: (tile, d_lo, d_hi)
    sched = []
    for t in range(NFULL):
        split = last_split if t == NFULL - 1 else tile_split
        d_lo = 0
        for w_d in split:
            sched.append((t, d_lo, d_lo + w_d))
            d_lo += w_d
        assert d_lo == D

    with tile.TileContext(nc) as tc:
        with (
            tc.tile_pool(name="const", bufs=1) as const_pool,
            tc.tile_pool(name="g", bufs=3) as g_pool,
            tc.tile_pool(name="rem", bufs=1) as rem_pool,
            tc.tile_pool(name="tps", bufs=3, space="PSUM") as tps_pool,
            tc.tile_pool(name="ops", bufs=4, space="PSUM") as ops_pool,
            tc.tile_pool(name="osb", bufs=4) as o_pool,
        ):
            # idx first on SP HWDGE: its ~2.9us completion chain gates every
            # gather's desc-gen, so its transfer must beat xt.
            idx_sb = const_pool.tile([128, IDX_COLS], i16)
            nc.sync.dma_start(idx_sb[:], idx[:])

            # xt second on SP: transfer hides inside the idx-chain window.
            xt_sb = const_pool.tile([128, DC * B], bf16)
            nc.sync.dma_start(xt_sb[:], xt[:])

            ident = const_pool.tile([128, 128], bf16)
            make_identity(nc, ident[:])

            for rep in range(reps):
                sfx = f"r{rep}"
                # --- remainder tile: exact-size row gather, rows on
                # partitions, then PE-transpose to d-major.
                wn = rem_pool.tile([128, 1, D], bf16, name=f"wn{sfx}")
                nc.gpsimd.dma_gather(
                    wn[:],
                    w[:],
                    idx_sb[:, NFULL * 8 : IDX_COLS],
                    rem,
                    rem,
                    D,
                    transpose=False,
                )
                g_rem = rem_pool.tile([128, DC * 128], bf16, name=f"gr{sfx}")
                for grp in range(8):
                    ps = tps_pool.tile([128, 512], bf16, tag="tps", name=f"tp{sfx}_{grp}")
                    for j in range(4):
                        c = grp * 4 + j
                        nc.tensor.transpose(
                            ps[:, j * 128 : (j + 1) * 128],
                            wn[:, 0, c * 128 : (c + 1) * 128],
                            ident[:],
                        )
                    nc.vector.tensor_copy(
                        g_rem[:, grp * 512 : (grp + 1) * 512], ps[:]
                    )

                ps_rem = ops_pool.tile([128, B], f32, tag="ops", name=f"pr{sfx}")
                for c in range(DC):
                    nc.tensor.matmul(
                        ps_rem[:],
                        lhsT=g_rem[:, c * 128 : (c + 1) * 128],
                        rhs=xt_sb[:, c * B : (c + 1) * B],
                        start=(c == 0),
                        stop=(c == DC - 1),
                    )
                o_rem = o_pool.tile([128, 1, B], f32, tag="ot", name=f"or{sfx}")
                nc.vector.tensor_copy(o_rem[:, 0, :], ps_rem[:])
                nc.sync.dma_start(
                    out[NFULL * 128 : NFULL * 128 + rem, :], o_rem[:rem, 0, :]
                )

                # --- full tiles: transposed gathers, D-split.
                g_tiles = {
                    t: g_pool.tile([128, DC * 128], bf16, tag="g", name=f"g{sfx}_{t}")
                    for t in range(NFULL)
                }
                psums = {}
                done = [0] * NFULL
                for t, lo, hi in sched:
                    g = g_tiles[t]
                    span = hi - lo
                    gsl = g[:, lo:hi].rearrange("p (s n) -> p s n", n=128)
                    nc.gpsimd.dma_gather(
                        gsl,
                        w[:, lo:hi],
                        idx_sb[:, t * 8 : (t + 1) * 8],
                        128,
                        128,
                        span,
                        elem_step=D,
                        transpose=True,
                    )
                    if done[t] == 0:
                        psums[t] = ops_pool.tile(
                            [128, B], f32, tag="ops", name=f"ps{sfx}_{t}"
                        )
                    for c in range(lo // 128, hi // 128):
                        nc.tensor.matmul(
                            psums[t][:],
                            lhsT=g[:, c * 128 : (c + 1) * 128],
                            rhs=xt_sb[:, c * B : (c + 1) * B],
                            start=(c == 0),
                            stop=(c == DC - 1),
                        )
                    done[t] += span // 128
                    if done[t] == DC:
                        ot = o_pool.tile([128, 1, B], f32, tag="ot", name=f"ot{sfx}_{t}")
                        nc.vector.tensor_copy(ot[:, 0, :], psums[t][:])
                        nc.sync.dma_start(
                            out[t * 128 : (t + 1) * 128, :], ot[:, 0, :]
                        )

    nc.compile()
    return nc


def _get_compiled(w_rows=W_ROWS, reps=1):
    key = (w_rows, reps)
    if key not in _compiled:
        _compiled[key] = _build(w_rows, reps)
    return _compiled[key]


def _wrap_idx16(ids):
    """[n*16] ints -> [128, n] int16: 16-wrapped, replicated 8x down."""
    n = ids.shape[0] // 16
    wrapped = ids.astype(np.int16).reshape(n, 16).T  # [16, n]
    return np.tile(wrapped, (8, 1))


def _prep_xt(x):
    x = np.asarray(x, dtype=np.float32).reshape(B, D)
    xv = x.astype(ml_dtypes.bfloat16)
    return np.ascontiguousarray(
        xv.T.reshape(DC, 128, B).transpose(1, 0, 2)
    ).reshape(128, DC * B)


def _prep_inputs(x, weight, indices):
    wbf = np.asarray(weight, dtype=np.float32).astype(ml_dtypes.bfloat16)
    indices = np.asarray(indices).astype(np.int64).reshape(N_IDX)
    xt_host = _prep_xt(x)

    uidx, inv = np.unique(indices, return_inverse=True)
    n_u = uidx.size

    q = -(-n_u // NCORES)  # ceil; per-core unique count
    assert q <= NPC, f"per-core count {q} exceeds NPC {NPC}"
    w_rows = W_ROWS

    in_maps = []
    counts = []
    for c in range(NCORES):
        lo, hi = c * q, min((c + 1) * q, n_u)
        cnt = hi - lo
        counts.append(cnt)
        ids = np.zeros(NPC, dtype=np.int64)
        ids[:cnt] = uidx[lo:hi]
        row_lo = int(ids[0])
        span = int(ids[cnt - 1]) - row_lo + 1
        if span > w_rows:
            w_rows = V  # pathological input; ship the full table
            row_lo = 0
        else:
            row_lo = min(row_lo, V - w_rows)
        ids[:cnt] -= row_lo
        ids[cnt:] = 0
        in_maps.append(
            {
                "w": np.ascontiguousarray(wbf[row_lo : row_lo + w_rows]),
                "xt": xt_host,
                "idx": np.ascontiguousarray(_wrap_idx16(ids)),
            }
        )

    return in_maps, counts, inv, w_rows


def _run(in_maps, w_rows, trace=False):
    from concourse.bass_utils import run_bass_kernel_spmd

    nc = _get_compiled(w_rows, 1)
    kw = {"trace": True} if trace else {}
    return run_bass_kernel_spmd(nc, in_maps, core_ids=list(range(NCORES)), **kw)


def kernel(x, weight, indices, _trace=False):
    in_maps, counts, inv, w_rows = _prep_inputs(x, weight, indices)
    res = _run(in_maps, w_rows, trace=_trace)

    parts = [res.results[c]["out"][: counts[c], :].T for c in range(NCORES)]
    y_unique = np.concatenate(parts, axis=1)  # [B, n_unique] sorted order
    out = np.ascontiguousarray(y_unique[:, inv]).reshape(B, 1, N_IDX)
    if _trace:
        return out, res
    return out
